# revision 3
# baseline (speedup 1.0000x reference)
import sys

import numpy as np

for p in ("/opt/trn_rl_repo",):
    if p not in sys.path:
        sys.path.insert(0, p)

import ml_dtypes  # noqa: E402

import concourse.tile as tile  # noqa: E402
from concourse import bacc, mybir  # noqa: E402
from concourse.bass_utils import run_bass_kernel_spmd  # noqa: E402

B, N, D = 128, 512, 512
NCORES = 8
BPC = B // NCORES  # 16 batch items per core
F32 = mybir.dt.float32
BF16 = mybir.dt.bfloat16
ACT_COPY = mybir.ActivationFunctionType.Copy


def _hadamard(n: int) -> np.ndarray:
    H = np.array([[1.0]], dtype=np.float32)
    base = np.array([[1.0, 1.0], [1.0, -1.0]], dtype=np.float32)
    while H.shape[0] < n:
        H = np.kron(H, base)
    return H


def _build():
    # y = H512 @ x @ H512 / 512 per item, bf16 device I/O.  DMA floor =
    # 16.8MB/core at 360B/ns = 46.6us; engines are balanced to ~3us/item.
    #
    # H512 = (H2 (x) I256)(I2 (x) H2 (x) I128)(I4 (x) H128); all factors
    # commute, so butterfly levels run pre-matmul on cheap bf16 SBUF ops:
    #   F1 row level (nblk pairs (0,2),(1,3))     - DVE
    #   G1 col level (d halves)                   - add DVE, sub Pool
    #   F2 row level (pairs (0,1),(2,3)): block 0 on DVE; blocks 1,2,3
    #     absorbed into K=256 left matmuls (2 PSUM-accum steps, H256 halves
    #     or +-H128) - trades cheap PE rows for scarce DVE throughput.
    # left PE  -> t^T PSUM [d-chunk, n] (4 banks)
    # middle eviction: Act 2 banks + DVE 2 banks -> tt bf16
    # right PE (K=256 vs H256/512 halves) -> y natural [n-chunk, e] (4 banks)
    # final eviction: Act one 4-bank op -> yt bf16 -> DMA out.
    #
    # Software pipeline per iteration k: load(k+3) | pre(k+1) | left(k+1)
    # trails pre by one | mid(k) | right(k-1), fin(k-1), store(k-1) - PE
    # alternates left/right of different items so it never waits on an
    # eviction; PSUM is exactly tp(4)+yp(4) banks with bufs=1 each.
    nc = bacc.Bacc("TRN2", target_bir_lowering=False, debug=False)
    x_d = nc.dram_tensor("x", [BPC, 4, 128, D], BF16, kind="ExternalInput").ap()
    hc_d = nc.dram_tensor("hc", [128, 2, 2, 256], BF16, kind="ExternalInput").ap()
    y_d = nc.dram_tensor("y", [BPC, 4, 128, D], BF16, kind="ExternalOutput").ap()

    with tile.TileContext(nc) as tc:
        with (
            tc.tile_pool(name="const", bufs=1) as cpool,
            tc.tile_pool(name="xp", bufs=5) as xpool,
            tc.tile_pool(name="xa", bufs=3) as apool,
            tc.tile_pool(name="xb", bufs=3) as bpool,
            tc.tile_pool(name="xc", bufs=3) as ccpool,
            tc.tile_pool(name="tt", bufs=2) as ttpool,
            tc.tile_pool(name="yt", bufs=2) as ytpool,
            tc.tile_pool(name="tp", bufs=1, space="PSUM") as tppool,
            tc.tile_pool(name="yp", bufs=1, space="PSUM") as yppool,
        ):
            hc = cpool.tile([128, 2, 2, 256], BF16)
            h256r = hc[:, 0]   # [128, 2, 256] rows of H256, split in halves
            hs256r = hc[:, 1]  # H256 / 512
            h128 = hc[:, 0, 0, 0:128]        # H256 = [[H,H],[H,-H]]
            h128n = hc[:, 0, 1, 128:256]     # -H128

            def stage_load(b):
                xt = xpool.tile([128, 4, D], BF16, tag="xt", name="xt")
                xsrc = x_d[b].transpose([1, 0, 2])
                if b == 0:
                    # Head: constants slot between the two halves of load 0.
                    hi, lo = slice(2, 4), slice(0, 2)
                    nc.sync.dma_start(xt[:, lo], xsrc[:, lo])
                    nc.sync.dma_start(hc[:], hc_d[:])
                    nc.sync.dma_start(xt[:, hi], xsrc[:, hi])
                else:
                    nc.sync.dma_start(xt[:], xsrc)
                return xt

            def stage_pre(xt):
                # F1 row level: pairs (0,2), (1,3)
                xa = apool.tile([128, 4, D], BF16, tag="xa", name="xa")
                nc.vector.tensor_add(xa[:, 0:2], xt[:, 0:2], xt[:, 2:4])
                nc.vector.tensor_sub(xa[:, 2:4], xt[:, 0:2], xt[:, 2:4])
                # G1 col level: d halves (sub on Pool to offload DVE)
                xb = bpool.tile([128, 4, D], BF16, tag="xb", name="xb")
                nc.vector.tensor_add(
                    xb[:, :, 0:256], xa[:, :, 0:256], xa[:, :, 256:512]
                )
                nc.gpsimd.tensor_sub(
                    xb[:, :, 256:512], xa[:, :, 0:256], xa[:, :, 256:512]
                )
                # F2 row level, output block 0 only (= xb0 + xb1); blocks
                # 1,2,3 are absorbed into the K=256 left matmuls.
                xc = ccpool.tile([128, 1, D], BF16, tag="xc", name="xc")
                nc.vector.tensor_add(xc[:, 0], xb[:, 0], xb[:, 1])
                return xb, xc

            def stage_left(xb, xc):
                tp = tppool.tile([128, 4, D], F32, tag="tp", name="tp")
                for j in range(4):
                    dsl = slice(128 * j, 128 * (j + 1))
                    # n-block 0: F2 pre-applied, K=128
                    nc.tensor.matmul(
                        tp[:, j, 0:128], xc[:, 0, dsl], h128,
                        start=True, stop=True,
                    )
                    # n-block 1 = xb0 - xb1: K=256 via +-H128
                    nc.tensor.matmul(
                        tp[:, j, 128:256], xb[:, 0, dsl], h128,
                        start=True, stop=False,
                    )
                    nc.tensor.matmul(
                        tp[:, j, 128:256], xb[:, 1, dsl], h128n,
                        start=False, stop=True,
                    )
                    # n-blocks 2,3: K=256 vs H256 row-halves
                    for s in range(2):
                        nc.tensor.matmul(
                            tp[:, j, 256:512], xb[:, 2 + s, dsl], h256r[:, s],
                            start=(s == 0), stop=(s == 1),
                        )
                return tp

            def stage_mid(tp):
                # PSUM -> SBUF eviction split: Act 2 banks, DVE 2 banks.
                tt = ttpool.tile([128, 4, D], BF16, tag="tt", name="tt")
                nc.scalar.activation(tt[:, 0:2], tp[:, 0:2], ACT_COPY)
                nc.vector.tensor_copy(tt[:, 2], tp[:, 2])
                nc.vector.tensor_copy(tt[:, 3], tp[:, 3])
                return tt

            def stage_right(tt):
                yp = yppool.tile([128, 4, D], F32, tag="yp", name="yp")
                for c in range(4):
                    nsl = slice(128 * c, 128 * (c + 1))
                    for h in range(2):
                        for s in range(2):
                            nc.tensor.matmul(
                                yp[:, c, 256 * h:256 * (h + 1)],
                                tt[:, 2 * h + s, nsl],
                                hs256r[:, s],
                                start=(s == 0), stop=(s == 1),
                            )
                return yp

            def stage_out(b, yp):
                yt = ytpool.tile([128, 4, D], BF16, tag="yt", name="yt")
                nc.scalar.activation(yt[:], yp[:], ACT_COPY)
                nc.sync.dma_start(y_d[b].transpose([1, 0, 2]), yt[:])

            # prologue
            xts, pres, tps, tts, yps = {}, {}, {}, {}, {}
            for b in range(min(3, BPC)):
                xts[b] = stage_load(b)
            pres[0] = stage_pre(xts.pop(0))
            # steady state: iteration k handles left(k), mid(k), pre(k+1),
            # right(k-1), fin(k-1), store(k-1), load(k+3)
            for k in range(BPC + 1):
                if k < BPC:
                    tps[k] = stage_left(*pres.pop(k))
                    tts[k] = stage_mid(tps.pop(k))
                    if k + 1 < BPC:
                        pres[k + 1] = stage_pre(xts.pop(k + 1))
                    if k + 3 < BPC:
                        xts[k + 3] = stage_load(k + 3)
                if k - 1 >= 0:
                    yp = stage_right(tts.pop(k - 1))
                    stage_out(k - 1, yp)

    nc.compile()
    return nc


_NC = None


def kernel(x: np.ndarray) -> np.ndarray:
    global _NC
    if _NC is None:
        _NC = _build()
    x = np.ascontiguousarray(
        np.asarray(x, dtype=np.float32).astype(ml_dtypes.bfloat16)
    )
    H = _hadamard(256)
    # hc[p, 0, s, q] = H256[s*128+p, q]; hc[p, 1, s, q] = same / 512
    hrows = H.reshape(2, 128, 256).transpose(1, 0, 2)  # [128, 2, 256]
    hc = np.stack([hrows, hrows / np.float32(512.0)], axis=1)
    hc = np.ascontiguousarray(hc.astype(ml_dtypes.bfloat16))
    xr = x.reshape(NCORES, BPC, 4, 128, D)
    in_maps = [{"x": xr[i], "hc": hc} for i in range(NCORES)]
    res = run_bass_kernel_spmd(_NC, in_maps, list(range(NCORES))).results
    return np.concatenate(
        [np.asarray(r["y"]).reshape(BPC, N, D) for r in res], axis=0
    ).astype(np.float32)


# revision 5
# speedup vs baseline: 1.0818x; 1.0818x over previous
import sys

import numpy as np

for p in ("/opt/trn_rl_repo",):
    if p not in sys.path:
        sys.path.insert(0, p)

import ml_dtypes  # noqa: E402

import concourse.tile as tile  # noqa: E402
from concourse import bacc, mybir  # noqa: E402
from concourse.bass_utils import run_bass_kernel_spmd  # noqa: E402

B, N, D = 128, 512, 512
NCORES = 8
BPC = B // NCORES  # 16 batch items per core
F32 = mybir.dt.float32
BF16 = mybir.dt.bfloat16
ACT_COPY = mybir.ActivationFunctionType.Copy


def _hadamard(n: int) -> np.ndarray:
    H = np.array([[1.0]], dtype=np.float32)
    base = np.array([[1.0, 1.0], [1.0, -1.0]], dtype=np.float32)
    while H.shape[0] < n:
        H = np.kron(H, base)
    return H


def _build():
    # y = H512 @ x @ H512 / 512 per item, bf16 device I/O.  DMA floor =
    # 16.8MB/core at 360B/ns = 46.6us; engines are balanced to ~3us/item.
    #
    # H512 = (H2 (x) I256)(I2 (x) H2 (x) I128)(I4 (x) H128); all factors
    # commute, so butterfly levels run pre-matmul on cheap bf16 SBUF ops:
    #   F1 row level (nblk pairs (0,2),(1,3))     - DVE
    #   G1 col level (d halves)                   - add DVE, sub Pool
    #   F2 row level (pairs (0,1),(2,3)): block 0 on DVE; blocks 1,2,3
    #     absorbed into K=256 left matmuls (2 PSUM-accum steps, H256 halves
    #     or +-H128) - trades cheap PE rows for scarce DVE throughput.
    # left PE  -> t^T PSUM [d-chunk, n] (4 banks)
    # middle eviction: Act 2 banks + DVE 2 banks -> tt bf16
    # right PE (K=256 vs H256/512 halves) -> y natural [n-chunk, e] (4 banks)
    # final eviction: Act one 4-bank op -> yt bf16 -> DMA out.
    #
    # Software pipeline per iteration k: load(k+3) | pre(k+1) | left(k+1)
    # trails pre by one | mid(k) | right(k-1), fin(k-1), store(k-1) - PE
    # alternates left/right of different items so it never waits on an
    # eviction; PSUM is exactly tp(4)+yp(4) banks with bufs=1 each.
    nc = bacc.Bacc("TRN2", target_bir_lowering=False, debug=False)
    x_d = nc.dram_tensor("x", [BPC, 4, 128, D], BF16, kind="ExternalInput").ap()
    hc_d = nc.dram_tensor("hc", [128, 2, 2, 256], BF16, kind="ExternalInput").ap()
    y_d = nc.dram_tensor("y", [BPC, 4, 128, D], BF16, kind="ExternalOutput").ap()

    with tile.TileContext(nc) as tc:
        with (
            tc.tile_pool(name="const", bufs=1) as cpool,
            tc.tile_pool(name="xp", bufs=5) as xpool,
            tc.tile_pool(name="xa", bufs=3) as apool,
            tc.tile_pool(name="xb", bufs=3) as bpool,
            tc.tile_pool(name="xc", bufs=3) as ccpool,
            tc.tile_pool(name="tt", bufs=2) as ttpool,
            tc.tile_pool(name="yt", bufs=2) as ytpool,
            tc.tile_pool(name="tp", bufs=1, space="PSUM") as tppool,
            tc.tile_pool(name="yp", bufs=1, space="PSUM") as yppool,
        ):
            hc = cpool.tile([128, 2, 2, 256], BF16)
            h256r = hc[:, 0]   # [128, 2, 256] rows of H256, split in halves
            hs256r = hc[:, 1]  # H256 / 512
            h128 = hc[:, 0, 0, 0:128]        # H256 = [[H,H],[H,-H]]
            h128n = hc[:, 0, 1, 128:256]     # -H128

            def stage_load(b):
                xt = xpool.tile([128, 4, D], BF16, tag="xt", name="xt")
                xsrc = x_d[b].transpose([1, 0, 2])
                if b == 0:
                    # Head: constants slot between the two halves of load 0.
                    hi, lo = slice(2, 4), slice(0, 2)
                    nc.sync.dma_start(xt[:, lo], xsrc[:, lo])
                    nc.sync.dma_start(hc[:], hc_d[:])
                    nc.sync.dma_start(xt[:, hi], xsrc[:, hi])
                else:
                    nc.sync.dma_start(xt[:], xsrc)
                return xt

            def stage_pre(xt):
                # F1 row level: pairs (0,2), (1,3)
                xa = apool.tile([128, 4, D], BF16, tag="xa", name="xa")
                nc.vector.tensor_add(xa[:, 0:2], xt[:, 0:2], xt[:, 2:4])
                nc.vector.tensor_sub(xa[:, 2:4], xt[:, 0:2], xt[:, 2:4])
                # G1 col level: d halves (sub on Pool to offload DVE)
                xb = bpool.tile([128, 4, D], BF16, tag="xb", name="xb")
                nc.vector.tensor_add(
                    xb[:, :, 0:256], xa[:, :, 0:256], xa[:, :, 256:512]
                )
                nc.gpsimd.tensor_sub(
                    xb[:, :, 256:512], xa[:, :, 0:256], xa[:, :, 256:512]
                )
                # F2 row level, output block 0 only (= xb0 + xb1); blocks
                # 1,2,3 are absorbed into the K=256 left matmuls.
                xc = ccpool.tile([128, 1, D], BF16, tag="xc", name="xc")
                nc.vector.tensor_add(xc[:, 0], xb[:, 0], xb[:, 1])
                return xb, xc

            def stage_left(xb, xc):
                tp = tppool.tile([128, 4, D], F32, tag="tp", name="tp")
                for j in range(4):
                    dsl = slice(128 * j, 128 * (j + 1))
                    # n-block 0: F2 pre-applied, K=128
                    nc.tensor.matmul(
                        tp[:, j, 0:128], xc[:, 0, dsl], h128,
                        start=True, stop=True,
                    )
                    # n-block 1 = xb0 - xb1: K=256 via +-H128
                    nc.tensor.matmul(
                        tp[:, j, 128:256], xb[:, 0, dsl], h128,
                        start=True, stop=False,
                    )
                    nc.tensor.matmul(
                        tp[:, j, 128:256], xb[:, 1, dsl], h128n,
                        start=False, stop=True,
                    )
                    # n-blocks 2,3: K=256 vs H256 row-halves
                    for s in range(2):
                        nc.tensor.matmul(
                            tp[:, j, 256:512], xb[:, 2 + s, dsl], h256r[:, s],
                            start=(s == 0), stop=(s == 1),
                        )
                return tp

            def stage_mid(tp):
                # PSUM -> SBUF eviction split: Act 2 banks, DVE 2 banks.
                tt = ttpool.tile([128, 4, D], BF16, tag="tt", name="tt")
                nc.scalar.activation(tt[:, 0:2], tp[:, 0:2], ACT_COPY)
                nc.vector.tensor_copy(tt[:, 2:4], tp[:, 2:4])
                return tt

            def stage_right(tt):
                yp = yppool.tile([128, 4, D], F32, tag="yp", name="yp")
                for c in range(4):
                    nsl = slice(128 * c, 128 * (c + 1))
                    for h in range(2):
                        for s in range(2):
                            nc.tensor.matmul(
                                yp[:, c, 256 * h:256 * (h + 1)],
                                tt[:, 2 * h + s, nsl],
                                hs256r[:, s],
                                start=(s == 0), stop=(s == 1),
                            )
                return yp

            def stage_out(b, yp):
                yt = ytpool.tile([128, 4, D], BF16, tag="yt", name="yt")
                nc.scalar.activation(yt[:], yp[:], ACT_COPY)
                nc.sync.dma_start(y_d[b].transpose([1, 0, 2]), yt[:])

            # prologue
            xts, pres, tps, tts, yps = {}, {}, {}, {}, {}
            for b in range(min(3, BPC)):
                xts[b] = stage_load(b)
            pres[0] = stage_pre(xts.pop(0))
            # steady state: iteration k handles left(k), mid(k), pre(k+1),
            # right(k-1), fin(k-1), store(k-1), load(k+3)
            for k in range(BPC + 1):
                if k < BPC:
                    # pre(k+1) first: its DVE ops precede mid(k)'s copies in
                    # the in-order DVE queue, so DVE works on pre(k+1) while
                    # PE runs left(k) instead of stalling on mid(k)'s dep.
                    if k + 1 < BPC:
                        pres[k + 1] = stage_pre(xts.pop(k + 1))
                    tps[k] = stage_left(*pres.pop(k))
                    tts[k] = stage_mid(tps.pop(k))
                    if k + 3 < BPC:
                        xts[k + 3] = stage_load(k + 3)
                if k - 1 >= 0:
                    yp = stage_right(tts.pop(k - 1))
                    stage_out(k - 1, yp)

    nc.compile()
    return nc


_NC = None


def kernel(x: np.ndarray) -> np.ndarray:
    global _NC
    if _NC is None:
        _NC = _build()
    x = np.ascontiguousarray(
        np.asarray(x, dtype=np.float32).astype(ml_dtypes.bfloat16)
    )
    H = _hadamard(256)
    # hc[p, 0, s, q] = H256[s*128+p, q]; hc[p, 1, s, q] = same / 512
    hrows = H.reshape(2, 128, 256).transpose(1, 0, 2)  # [128, 2, 256]
    hc = np.stack([hrows, hrows / np.float32(512.0)], axis=1)
    hc = np.ascontiguousarray(hc.astype(ml_dtypes.bfloat16))
    xr = x.reshape(NCORES, BPC, 4, 128, D)
    in_maps = [{"x": xr[i], "hc": hc} for i in range(NCORES)]
    res = run_bass_kernel_spmd(_NC, in_maps, list(range(NCORES))).results
    return np.concatenate(
        [np.asarray(r["y"]).reshape(BPC, N, D) for r in res], axis=0
    ).astype(np.float32)
